# revision 31
# baseline (speedup 1.0000x reference)
"""Trainium2 Bass kernel for a dense multi-head attention block.

Full (unsharded) contract: kernel(**inputs) -> np.ndarray [2, 2048, 1024].

Sharding: 8 cores = 2 (batch) x 4 (head-group of 4 heads).  Each core
computes Q/K/V projections for its 4 heads, RoPE, causal attention, and
a partial output (attn_group @ wo_rows).  The 4 partials per batch are
summed on the host (the tensor-parallel unshard).

v3 design: everything f16 on device (f32 PSUM accumulation), organized
around keeping the PE engine continuously busy (the tensor engine only
reaches its 2.4 GHz p-state after ~3us of uninterrupted execution, so
every stall is doubly costly).  PSUM is statically partitioned into 8
banks that stage1 (projections+RoPE for chunk c+1) and stage2 (causal
attention for chunk c) own concurrently:
  scr0, scr1 : [P,2,SC] score pairs   (4 banks, drained by ACT exp)
  pv0,  pv1  : [P,SC]   PV accum      (2 banks, row 64 = softmax denom)
  s1qk       : [P,SC]   q/k proj + rope swap  (1 bank)
  misc       : [P,SC]   v proj + wo           (1 bank)
Emission interleaves score pairs / projection steps / PV / wo at sub-us
granularity so the in-order PE queue always has ready work while ACT and
DVE drain behind it.  The causal mask is applied as a multiplicative 0/1
f16 mask on the exp arena (cheap 4x-mode DVE op) instead of additive
-1e9 adds on f32 PSUM.  Softmax denominators come out of the PV matmul
as PSUM row 64 (ones column appended to V); 1/x uses the fast custom-DVE
reciprocal directly on that PSUM row, broadcast to 64 partitions via the
gpsimd partition_broadcast, and multiplied into attnT in-place.
"""

import os
import sys
import types

import numpy as np

B, S, D, H = 2, 2048, 1024, 16
HD = D // H          # 64
NHG = 4              # head-groups (tensor-parallel dim)
NH = 4               # heads per core
DHG = 256            # head dims per core
P = 128
N_CORES = 8
KT = D // P          # 8 contraction tiles for the projections
NST = S // P         # 16 sequence tiles
SC = 512             # sequence chunk (pipeline granularity)
NSC = S // SC        # 4 chunks

_CACHE = {}


def _install_trace_shim():
    """Make antenv.axon_hooks importable so bass_utils trace=True works."""
    if "antenv.axon_hooks" in sys.modules:
        return
    try:
        import trn_agent_boot.trn_boot as _tb
        hook = _tb._ntff_profile_via_ctypes("/opt/axon/libaxon_pjrt.so")
    except Exception:
        hook = None
    mod = types.ModuleType("antenv.axon_hooks")
    mod.get_axon_ntff_profile_hook = lambda: hook
    mod.set_axon_ntff_profile_hook = lambda h: None
    sys.modules["antenv.axon_hooks"] = mod


def _weave(*streams):
    """Proportionally interleave lists of thunks and run them in order."""
    streams = [list(s) for s in streams if s]
    if not streams:
        return
    total = max(len(s) for s in streams)
    idx = [0.0] * len(streams)
    steps = [len(s) / total for s in streams]
    for _ in range(total):
        for si, s in enumerate(streams):
            idx[si] += steps[si]
            while idx[si] >= 1.0 and s:
                s.pop(0)()
                idx[si] -= 1.0
    for s in streams:
        for th in s:
            th()


def _emit(tc, nc, ap, out_ap, mybir, dbg=None):
    from contextlib import ExitStack

    f32 = mybir.dt.float32
    f16 = mybir.dt.float16
    Exp = mybir.ActivationFunctionType.Exp

    with ExitStack() as ctx:
        consts = ctx.enter_context(tc.tile_pool(name="consts", bufs=1))

        wo_sb = consts.tile([P, 2, D], f16)
        pm_sb = consts.tile([P, P], f16)
        tri4 = consts.tile([P, 4, P], f16)     # 0/1 causal mask, 4 copies
        # per-chunk tiles so dependency tracking stays exact (no false
        # serialization of old-chunk reads behind new-chunk writes)
        qrot = [consts.tile([P, 2, SC], f16, name=f"qrot{c}") for c in range(NSC)]
        krot = [consts.tile([P, 2, SC], f16, name=f"krot{c}") for c in range(NSC)]
        v65 = [consts.tile([P, 4, NH, HD + 1], f16, name=f"v65_{c}")
               for c in range(NSC)]
        for c in range(NSC):
            nc.vector.memset(v65[c][:, :, :, HD:HD + 1], 1.0)
        xT_sb = [consts.tile([P, KT, SC], f16, name=f"xT{c}") for c in range(NSC)]
        w_sb = {}
        for wn in ("wq", "wk", "wv"):
            w_sb[wn] = consts.tile([P, KT, DHG], f16, name=f"w_{wn}")
        cs_sb = {}
        for cn in ("ccq", "ssq", "cck", "ssk"):
            cs_sb[cn] = consts.tile([P, S], f16, name=f"cs_{cn}")
        arena = {}
        for h in range(NH):
            # one spare slot so the strided causal-mask band AP stays in range
            arena[h] = consts.tile([P, NST + 1, SC], f16, name=f"ar{h}")
        ones4 = consts.tile([P, 64], f16)
        nc.vector.memset(ones4, 1.0)
        den4 = consts.tile([P, SC], f32)
        nc.vector.memset(den4, 1.0)
        rcp4 = consts.tile([P, SC], f16)
        rcp_h3 = consts.tile([1, SC], f16)
        lnt = consts.tile([P, SC], f32)

        tmp = ctx.enter_context(tc.tile_pool(name="tmp", bufs=1))
        attnT_p = ctx.enter_context(tc.tile_pool(name="attnT", bufs=2))
        obp = ctx.enter_context(tc.tile_pool(name="obp", bufs=4))
        ps = ctx.enter_context(tc.tile_pool(name="ps", bufs=1, space="PSUM"))

        xTr = ap["xT"].rearrange("(a p) s -> p a s", p=P)
        wre = {wn: ap[wn].rearrange("(a p) m -> p a m", p=P)
               for wn in ("wq", "wk", "wv")}

        def dma_x_chunk(sc, eng):
            ssl = slice(sc * SC, (sc + 1) * SC)
            eng.dma_start(out=xT_sb[sc], in_=xTr[:, :, ssl])

        # ---- input DMAs: the first-needed tensors in fine pieces, split
        # across both rings so the prologue's a-loop is paced by arrivals.
        rings = (nc.gpsimd, nc.sync)
        for piece in range(4):
            asl = slice(2 * piece, 2 * piece + 2)
            rings[piece % 2].dma_start(out=w_sb["wq"][:, asl, :],
                                       in_=wre["wq"][:, asl, :])
            rings[(piece + 1) % 2].dma_start(out=xT_sb[0][:, asl, :],
                                             in_=xTr[:, asl, 0:SC])
        nc.gpsimd.dma_start(out=w_sb["wk"][:, 0:KT // 2, :],
                            in_=wre["wk"][:, 0:KT // 2, :])
        nc.sync.dma_start(out=w_sb["wk"][:, KT // 2:, :],
                          in_=wre["wk"][:, KT // 2:, :])
        nc.gpsimd.dma_start(out=w_sb["wv"], in_=wre["wv"])
        nc.sync.dma_start(out=pm_sb, in_=ap["pm"])
        for cn in ("ccq", "ssq", "cck", "ssk"):
            nc.gpsimd.dma_start(out=cs_sb[cn], in_=ap[cn])
        nc.sync.dma_start(out=tri4, in_=ap["tri4"].rearrange("p (a c) -> p a c", a=4))
        dma_x_chunk(1, nc.sync)
        nc.sync.dma_start(out=wo_sb, in_=ap["wo"].rearrange("(a p) m -> p a m", p=P))

        qk_specs = (("wq", "ccq", "ssq", qrot), ("wk", "cck", "ssk", krot))
        evs = {}

        # ---- stage1 pieces ------------------------------------------------
        def qk_fill(sc, wn, m, tag):
            pr = ps.tile([P, SC], f32, tag=tag, name=f"pr_{wn}{m}_{sc}")
            for a in range(KT):
                nc.tensor.matmul(
                    pr, w_sb[wn][:, a, m * P:(m + 1) * P], xT_sb[sc][:, a, :],
                    start=(a == 0), stop=(a == KT - 1))
            return pr

        def qk_drain(sc, wn, m, pr):
            ev = tmp.tile([P, SC], f16, tag=f"ev{wn}{m}", name=f"ev_{wn}{m}_{sc}")
            nc.vector.tensor_copy(ev, pr)
            evs[(wn, m)] = ev

        def sw_rope(sc, wn, m, tag):
            ssl = slice(sc * SC, (sc + 1) * SC)
            ccn, ssn, rot = {
                "wq": ("ccq", "ssq", qrot), "wk": ("cck", "ssk", krot)}[wn]
            ev = evs[(wn, m)]
            sw = ps.tile([P, SC], f32, tag=tag, name=f"sw_{wn}{m}_{sc}")
            nc.tensor.matmul(sw, pm_sb, ev, start=True, stop=True)
            t2 = tmp.tile([P, SC], f16, tag="t2", name=f"t2_{wn}{m}_{sc}", bufs=2)
            nc.vector.tensor_mul(t2, sw, cs_sb[ssn][:, ssl])
            t1 = tmp.tile([P, SC], f16, tag="t1", name=f"t1_{wn}{m}_{sc}", bufs=2)
            nc.gpsimd.tensor_mul(t1, ev, cs_sb[ccn][:, ssl])
            nc.vector.tensor_add(rot[sc][:, m, :], t1, t2)

        def v_fill(sc, st, tag):
            vp = ps.tile([P, DHG], f32, tag=tag, name=f"vp{sc}_{st}")
            for a in range(KT):
                nc.tensor.matmul(
                    vp, xT_sb[sc][:, a, st * P:(st + 1) * P], w_sb["wv"][:, a, :],
                    start=(a == 0), stop=(a == KT - 1))
            return vp

        def v_drain(sc, st, vp):
            nc.vector.tensor_copy(v65[sc][:, st, :, 0:HD],
                                  vp.rearrange("p (h d) -> p h d", h=NH))

        # ---- stage2 pieces ------------------------------------------------
        def sc2_unit(qc, t, ip):
            """Scores + exp for key tiles (ip, ip+1), heads 2t and 2t+1."""
            i0, i1 = ip, ip + 1
            start = max(i0 * P - qc * SC, 0)
            scrs = {}
            for h in (2 * t, 2 * t + 1):
                scrs[h] = ps.tile([P, 2, SC], f32, tag=f"scr{h % 2}",
                                  name=f"scr{h}_{qc}_{ip}")
            for j, i in ((0, i0), (1, i1)):
                for h in (2 * t, 2 * t + 1):
                    po = 64 * (h % 2)
                    nc.tensor.matmul(
                        scrs[h][:, j, start:SC],
                        krot[i // 4][po:po + 64, t, (i % 4) * P:(i % 4 + 1) * P],
                        qrot[qc][po:po + 64, t, start:SC],
                        start=True, stop=True)
            for h in (2 * t, 2 * t + 1):
                nc.scalar.activation(
                    arena[h][:, i0:i0 + 2, start:SC],
                    scrs[h][:, :, start:SC], Exp)

        def trimask(qc, h):
            """Zero the 4 causal triangles of chunk qc in head h's arena."""
            flat = arena[h].rearrange("p a c -> p (a c)")
            band = flat[:, 2048 * qc:2048 * qc + 2560]
            band = band.rearrange("p (a c) -> p a c", a=4)[:, :, 0:P]
            nc.gpsimd.tensor_mul(band, band, tri4)

        attnTs = {}

        def pv_fill(qc, h, lo, hi):
            nt = 4 * qc + 4
            pv = ps.tile([P, SC], f32, tag=f"pv{h % 2}", name=f"pv{h}_{qc}")
            for i in range(lo, hi):
                scol = max(i * P - qc * SC, 0)
                nc.tensor.matmul(
                    pv[0:65, scol:SC], v65[i // 4][:, i % 4, h, :],
                    arena[h][:, i, scol:SC],
                    start=(i == 0), stop=(i == nt - 1))
            return pv

        def pv_drain(qc, h, pv):
            attnT = attnTs[qc]
            nc.vector.tensor_copy(
                attnT[64 * (h % 2):64 * (h % 2) + 64, h // 2, :], pv[0:64, :])
            nc.scalar.copy(den4[32 * h:32 * h + 1, :], pv[64:65, :])

        def recip(qc):
            with nc.allow_low_precision(reason="f16 softmax denominators"):
                nc.vector.reciprocal(rcp4, den4)
            nc.vector.tensor_copy(rcp_h3, rcp4[96:97, :])

        def norm(qc, t):
            bcp = ps.tile([P, SC], f32, tag="misc", name=f"bcp{t}_{qc}")
            attnT = attnTs[qc]
            for par in range(2):
                h = 2 * t + par
                if h == 3:
                    lhs, rhs = ones4[0:1, :], rcp_h3
                else:
                    lhs = ones4[32 * h:32 * h + 1, :]
                    rhs = rcp4[32 * h:32 * h + 1, :]
                nc.tensor.matmul(bcp[64 * par:64 * par + 64, :],
                                 lhs, rhs, start=True, stop=True)
            for par in range(2):
                h = 2 * t + par
                sl = slice(64 * par, 64 * par + 64)
                nc.vector.tensor_mul(attnT[sl, t, :], attnT[sl, t, :], bcp[sl, :])

        def wo_unit(qc, ml, nn, tag):
            attnT = attnTs[qc]
            m = qc * (SC // P) + ml
            wo_ps = ps.tile([P, SC], f32, tag=tag, name=f"wo{m}_{nn}")
            for kd in range(2):
                nc.tensor.matmul(
                    wo_ps, attnT[:, kd, ml * P:(ml + 1) * P],
                    wo_sb[:, kd, nn * 512:(nn + 1) * 512],
                    start=(kd == 0), stop=(kd == 1))
            ob = obp.tile([P, 512], f16, tag="ob", name=f"ob{m}_{nn}")
            if nn == 0:
                nc.vector.tensor_copy(ob, wo_ps)
            else:
                nc.scalar.copy(ob, wo_ps)
            eng = (nc.sync, nc.gpsimd)[(ml + nn) % 2]
            eng.dma_start(
                out=out_ap[m * P:(m + 1) * P, nn * 512:(nn + 1) * 512], in_=ob)

        # ---- prologue: stage1(chunk 0), DMA-paced ------------------------
        # wq's two accumulation groups run a-tile-major (paced by the wq+x0
        # transfers), then wk's (paced by the wk transfer), on the score/pv
        # banks.  Swaps and the v projection follow on the stage1 banks.
        pros = {}
        for wn, tag0, tag1 in (("wq", "scr0", "scr1"), ("wk", "pv0", "pv1")):
            for m, tag in ((0, tag0), (1, tag1)):
                pros[(wn, m)] = ps.tile([P, SC], f32, tag=tag,
                                        name=f"pro_{wn}{m}")
            for a in range(KT):
                for m in range(2):
                    nc.tensor.matmul(
                        pros[(wn, m)], w_sb[wn][:, a, m * P:(m + 1) * P],
                        xT_sb[0][:, a, :], start=(a == 0), stop=(a == KT - 1))
            for m in range(2):
                qk_drain(0, wn, m, pros[(wn, m)])
        # rope swaps, then the chunk-0 v projection woven with the first
        # score pairs of chunk 0 (they only wait on the rope DVE adds)
        sw_rope(0, "wq", 0, "misc")
        sw_rope(0, "wq", 1, "s1qk")
        v_drain(0, 0, v_fill(0, 0, "misc"))
        sw_rope(0, "wk", 0, "s1qk")
        v_drain(0, 1, v_fill(0, 1, "misc"))
        sw_rope(0, "wk", 1, "s1qk")

        # ---- main phases --------------------------------------------------
        def s1_units(c):
            """Three role-based unit lists: q-side rope, k-side rope, v.
            The k rope must be complete by the end of part 2 so the next
            phase's first score matmuls never wait on it."""
            if c >= NSC:
                return [], [], []
            ua, ub, uc = [], [], []
            if c + 1 < NSC:
                ua.append(lambda c=c: dma_x_chunk(c + 1, nc.gpsimd))
            for wn, m, u in (("wq", 0, ua), ("wq", 1, ua),
                             ("wk", 0, ub), ("wk", 1, ub)):
                u.append(lambda c=c, wn=wn, m=m: qk_drain(
                    c, wn, m, qk_fill(c, wn, m, "s1qk")))
                u.append(lambda c=c, wn=wn, m=m: sw_rope(c, wn, m, "s1qk"))
            for st in range(4):
                uc.append(lambda c=c, st=st: v_drain(
                    c, st, v_fill(c, st, "misc")))
            return ua, ub, uc

        def sc2_units(qc, t):
            nt = 4 * qc + 4
            units = [lambda qc=qc, t=t, ip=ip: sc2_unit(qc, t, ip)
                     for ip in range(0, nt, 2)]
            units.append(lambda qc=qc, h=2 * t: trimask(qc, h))
            units.append(lambda qc=qc, h=2 * t + 1: trimask(qc, h))
            return units

        def pv_units(qc, t):
            nt = 4 * qc + 4
            units = []
            for h in (2 * t, 2 * t + 1):
                bnds = list(range(0, nt, 6)) + [nt]
                segs = [(bnds[i], bnds[i + 1]) for i in range(len(bnds) - 1)]
                holder = {}

                def seg_run(qc=qc, h=h, seg=None, holder=holder, last=False):
                    pv = holder.get("pv")
                    if pv is None:
                        pv = pv_fill(qc, h, seg[0], seg[1])
                        holder["pv"] = pv
                    else:
                        nt_ = 4 * qc + 4
                        for i in range(seg[0], seg[1]):
                            scol = max(i * P - qc * SC, 0)
                            nc.tensor.matmul(
                                pv[0:65, scol:SC], v65[i // 4][:, i % 4, h, :],
                                arena[h][:, i, scol:SC],
                                start=False, stop=(i == nt_ - 1))
                    if last:
                        pv_drain(qc, h, pv)
                for si, seg in enumerate(segs):
                    units.append(lambda seg=seg, f=seg_run,
                                 last=(si == len(segs) - 1): f(
                                     seg=seg, last=last))
            return units

        def wo_units(fc, tags=("misc",)):
            units = []
            for k, (ml, nn) in enumerate(
                    (ml, nn) for ml in range(4) for nn in range(2)):
                units.append(lambda fc=fc, ml=ml, nn=nn,
                             tag=tags[k % len(tags)]: wo_unit(fc, ml, nn, tag))
            return units

        # chunk-0 t=0 scores run in the prologue; thereafter each phase p
        # handles [scores(p,1); pv(p,0); pv(p,1); scores(p+1,0)] so the
        # exp/ACT load stays spread across phase boundaries.
        attnTs[0] = attnT_p.tile([P, 2, SC], f16, tag="at", name="at0")
        _weave(sc2_units(0, 0),
               [lambda: v_drain(0, 2, v_fill(0, 2, "misc")),
                lambda: v_drain(0, 3, v_fill(0, 3, "s1qk"))])

        for p in range(NSC):
            qc, c, fc = p, p + 1, p - 1
            if c < NSC:
                attnTs[c] = attnT_p.tile([P, 2, SC], f16, tag="at", name=f"at{c}")
            s1a, s1b, s1c = s1_units(c)
            if fc >= 0:
                wo = [lambda fc=fc: recip(fc),
                      lambda fc=fc: norm(fc, 0),
                      lambda fc=fc: norm(fc, 1)]
                wo += wo_units(fc, tags=("misc",) if c < NSC else ("misc", "s1qk"))
            else:
                wo = []
            nwo = len(wo)
            woa, wob, woc = wo[:5], wo[5:8], wo[8:]
            tail = []
            if p == NSC - 1:
                # start normalizing the last chunk's first head-pair while
                # its second pair is still in PV, to shorten the tail
                def recip_t0(qc=qc):
                    with nc.allow_low_precision(reason="f16 softmax denoms"):
                        nc.vector.reciprocal(rcp4[0:64, :], den4[0:64, :])
                tail = [recip_t0, lambda qc=qc: norm(qc, 0)]
            _weave(sc2_units(qc, 1), s1a, woa)
            _weave(pv_units(qc, 0), s1b, wob)
            _weave(pv_units(qc, 1),
                   sc2_units(c, 0) if c < NSC else [],
                   s1c, woc + tail)

        # ---- epilogue: ACT-based reciprocal (exp(-ln)) for low latency ----
        Ln = mybir.ActivationFunctionType.Ln
        nc.scalar.activation(lnt[64:128, :], den4[64:128, :], Ln)
        with nc.allow_low_precision(reason="f16 softmax denominators"):
            nc.scalar.activation(rcp4[64:128, :], lnt[64:128, :], Exp, scale=-1.0)
        nc.vector.tensor_copy(rcp_h3, rcp4[96:97, :])
        norm(NSC - 1, 1)
        for th in wo_units(NSC - 1, tags=("misc", "s1qk")):
            th()

        if dbg is not None:
            for c in range(NSC):
                nc.sync.dma_start(out=dbg["qrot"][:, :, c * SC:(c + 1) * SC],
                                  in_=qrot[c])
                nc.sync.dma_start(out=dbg["krot"][:, :, c * SC:(c + 1) * SC],
                                  in_=krot[c])
                nc.sync.dma_start(out=dbg["v65"][:, 4 * c:4 * c + 4], in_=v65[c])
            nc.sync.dma_start(out=dbg["ar0"], in_=arena[0][:, 0:NST, :])
            nc.sync.dma_start(out=dbg["at3"], in_=attnTs[3])


def _build_program(debug=False):
    import concourse.tile as tile
    import concourse.mybir as mybir
    from concourse import bacc

    f16 = mybir.dt.float16

    nc = bacc.Bacc("TRN2", target_bir_lowering=False, debug=False,
                   num_devices=N_CORES)
    ap = {}

    def inp(name, shape, dt):
        ap[name] = nc.dram_tensor(name, shape, dt, kind="ExternalInput").ap()

    inp("xT", [D, S], f16)
    inp("wq", [D, DHG], f16)
    inp("wk", [D, DHG], f16)
    inp("wv", [D, DHG], f16)
    inp("wo", [DHG, D], f16)
    inp("ccq", [P, S], f16)
    inp("ssq", [P, S], f16)
    inp("cck", [P, S], f16)
    inp("ssk", [P, S], f16)
    inp("tri4", [P, 4 * P], f16)
    inp("pm", [P, P], f16)
    out_ap = nc.dram_tensor("out", [S, D], f16, kind="ExternalOutput").ap()
    dbg = None
    if debug:
        dbg = {
            "qrot": nc.dram_tensor("dbg_qrot", [P, 2, S], f16, kind="ExternalOutput").ap(),
            "krot": nc.dram_tensor("dbg_krot", [P, 2, S], f16, kind="ExternalOutput").ap(),
            "v65": nc.dram_tensor("dbg_v65", [P, NST, NH, HD + 1], f16, kind="ExternalOutput").ap(),
            "ar0": nc.dram_tensor("dbg_ar0", [P, NST, SC], f16, kind="ExternalOutput").ap(),
            "at3": nc.dram_tensor("dbg_at3", [P, 2, SC], f16, kind="ExternalOutput").ap(),
        }

    with tile.TileContext(nc) as tc:
        _emit(tc, nc, ap, out_ap, mybir, dbg=dbg)
    nc.compile()
    return nc


def _host_prep(x, wq, wk, wv, wo, freqs_cos, freqs_sin, mask):
    """Build the 8 per-core input maps."""
    perm = []
    for h in range(NH):
        perm += [HD * h + 2 * j for j in range(HD // 2)]
        perm += [HD * h + 2 * j + 1 for j in range(HD // 2)]
    perm = np.asarray(perm)

    cosT = np.ascontiguousarray(freqs_cos.T).astype(np.float32)   # [32, S]
    sinT = np.ascontiguousarray(freqs_sin.T).astype(np.float32)
    CC = np.tile(cosT, (4, 1))                                    # [128, S]
    SS = np.tile(np.vstack([-sinT, sinT]), (2, 1))                # [128, S]
    ccq, ssq = (CC * 0.125).astype(np.float16), (SS * 0.125).astype(np.float16)
    cck, ssk = CC.astype(np.float16), SS.astype(np.float16)

    swap = np.zeros((P, P), dtype=np.float16)
    for g in range(2):
        for j in range(32):
            swap[64 * g + 32 + j, 64 * g + j] = 1.0
            swap[64 * g + j, 64 * g + 32 + j] = 1.0

    m2 = mask[0, 0]
    # multiplicative 0/1 mask in [key, query] orientation, replicated 4x
    tri = (m2[0:P, 0:P].T > -0.5).astype(np.float16)
    tri4 = np.tile(tri, (1, 4)).astype(np.float16)

    xT = [np.ascontiguousarray(x[b].T).astype(np.float16) for b in range(B)]

    in_maps = []
    for c in range(N_CORES):
        b, hg = c // NHG, c % NHG
        cols = hg * DHG + np.arange(DHG)
        in_maps.append({
            "xT": xT[b],
            "wq": np.ascontiguousarray(wq[:, hg * DHG + perm]).astype(np.float16),
            "wk": np.ascontiguousarray(wk[:, hg * DHG + perm]).astype(np.float16),
            "wv": np.ascontiguousarray(wv[:, cols]).astype(np.float16),
            "wo": np.ascontiguousarray(wo[cols, :]).astype(np.float16),
            "ccq": ccq, "ssq": ssq, "cck": cck, "ssk": ssk,
            "tri4": tri4, "pm": swap,
        })
    return in_maps


def kernel(x, wq, wk, wv, wo, freqs_cos, freqs_sin, mask, start_pos=0, **_):
    import concourse.bass_utils as bass_utils

    x = np.asarray(x, dtype=np.float32)
    wq = np.asarray(wq, dtype=np.float32)
    wk = np.asarray(wk, dtype=np.float32)
    wv = np.asarray(wv, dtype=np.float32)
    wo = np.asarray(wo, dtype=np.float32)
    freqs_cos = np.asarray(freqs_cos, dtype=np.float32)
    freqs_sin = np.asarray(freqs_sin, dtype=np.float32)
    mask = np.asarray(mask, dtype=np.float32)

    trace = bool(int(os.environ.get("BASS_KERNEL_TRACE", "0")))
    if trace:
        _install_trace_shim()
        import concourse.bass_utils as bu
        bu.upload_artifacts = lambda tmpdir: "(upload skipped)"

    if "nc" not in _CACHE:
        _CACHE["nc"] = _build_program()
    nc = _CACHE["nc"]

    in_maps = _host_prep(x, wq, wk, wv, wo, freqs_cos, freqs_sin, mask)
    kwargs = {}
    if trace:
        kwargs = dict(trace=True, trace_cores=[0],
                      tmpdir=os.environ.get("BASS_KERNEL_TRACE_DIR", None))
    res = None
    last_exc = None
    for attempt in range(5):
        try:
            res = bass_utils.run_bass_kernel_spmd(
                nc, in_maps, core_ids=list(range(N_CORES)), **kwargs)
            break
        except Exception as e:  # transient NRT device errors recover on retry
            last_exc = e
            import time as _time
            _time.sleep(12)
    if res is None:
        raise last_exc
    _CACHE["last_result"] = res

    out = np.zeros((B, S, D), dtype=np.float32)
    for c in range(N_CORES):
        out[c // NHG] += res.results[c]["out"].astype(np.float32)
    return out


# revision 33
# speedup vs baseline: 1.0400x; 1.0400x over previous
"""Trainium2 Bass kernel for a dense multi-head attention block.

Full (unsharded) contract: kernel(**inputs) -> np.ndarray [2, 2048, 1024].

Sharding: 8 cores = 2 (batch) x 4 (head-group of 4 heads).  Each core
computes Q/K/V projections for its 4 heads, RoPE, causal attention, and
a partial output (attn_group @ wo_rows).  The 4 partials per batch are
summed on the host (the tensor-parallel unshard).

v3 design: everything f16 on device (f32 PSUM accumulation), organized
around keeping the PE engine continuously busy (the tensor engine only
reaches its 2.4 GHz p-state after ~3us of uninterrupted execution, so
every stall is doubly costly).  PSUM is statically partitioned into 8
banks that stage1 (projections+RoPE for chunk c+1) and stage2 (causal
attention for chunk c) own concurrently:
  scr0, scr1 : [P,2,SC] score pairs   (4 banks, drained by ACT exp)
  pv0,  pv1  : [P,SC]   PV accum      (2 banks, row 64 = softmax denom)
  s1qk       : [P,SC]   q/k proj + rope swap  (1 bank)
  misc       : [P,SC]   v proj + wo           (1 bank)
Emission interleaves score pairs / projection steps / PV / wo at sub-us
granularity so the in-order PE queue always has ready work while ACT and
DVE drain behind it.  The causal mask is applied as a multiplicative 0/1
f16 mask on the exp arena (cheap 4x-mode DVE op) instead of additive
-1e9 adds on f32 PSUM.  Softmax denominators come out of the PV matmul
as PSUM row 64 (ones column appended to V); 1/x uses the fast custom-DVE
reciprocal directly on that PSUM row, broadcast to 64 partitions via the
gpsimd partition_broadcast, and multiplied into attnT in-place.
"""

import os
import sys
import types

import numpy as np

B, S, D, H = 2, 2048, 1024, 16
HD = D // H          # 64
NHG = 4              # head-groups (tensor-parallel dim)
NH = 4               # heads per core
DHG = 256            # head dims per core
P = 128
N_CORES = 8
KT = D // P          # 8 contraction tiles for the projections
NST = S // P         # 16 sequence tiles
SC = 512             # sequence chunk (pipeline granularity)
NSC = S // SC        # 4 chunks

_CACHE = {}


def _install_trace_shim():
    """Make antenv.axon_hooks importable so bass_utils trace=True works."""
    if "antenv.axon_hooks" in sys.modules:
        return
    try:
        import trn_agent_boot.trn_boot as _tb
        hook = _tb._ntff_profile_via_ctypes("/opt/axon/libaxon_pjrt.so")
    except Exception:
        hook = None
    mod = types.ModuleType("antenv.axon_hooks")
    mod.get_axon_ntff_profile_hook = lambda: hook
    mod.set_axon_ntff_profile_hook = lambda h: None
    sys.modules["antenv.axon_hooks"] = mod


def _weave(*streams):
    """Proportionally interleave lists of thunks and run them in order."""
    streams = [list(s) for s in streams if s]
    if not streams:
        return
    total = max(len(s) for s in streams)
    idx = [0.0] * len(streams)
    steps = [len(s) / total for s in streams]
    for _ in range(total):
        for si, s in enumerate(streams):
            idx[si] += steps[si]
            while idx[si] >= 1.0 and s:
                s.pop(0)()
                idx[si] -= 1.0
    for s in streams:
        for th in s:
            th()


def _emit(tc, nc, ap, out_ap, mybir, dbg=None):
    from contextlib import ExitStack

    f32 = mybir.dt.float32
    f16 = mybir.dt.float16
    Exp = mybir.ActivationFunctionType.Exp

    with ExitStack() as ctx:
        consts = ctx.enter_context(tc.tile_pool(name="consts", bufs=1))

        wo_sb = consts.tile([P, 2, D], f16)
        pm_sb = consts.tile([P, P], f16)
        tri4 = consts.tile([P, 4, P], f16)     # 0/1 causal mask, 4 copies
        # per-chunk tiles so dependency tracking stays exact (no false
        # serialization of old-chunk reads behind new-chunk writes)
        qrot = [consts.tile([P, 2, SC], f16, name=f"qrot{c}") for c in range(NSC)]
        krot = [consts.tile([P, 2, SC], f16, name=f"krot{c}") for c in range(NSC)]
        v65 = [consts.tile([P, 4, NH, HD + 1], f16, name=f"v65_{c}")
               for c in range(NSC)]
        for c in range(NSC):
            nc.vector.memset(v65[c][:, :, :, HD:HD + 1], 1.0)
        xT_sb = [consts.tile([P, KT, SC], f16, name=f"xT{c}") for c in range(NSC)]
        w_sb = {}
        for wn in ("wq", "wk", "wv"):
            w_sb[wn] = consts.tile([P, KT, DHG], f16, name=f"w_{wn}")
        cs_sb = {}
        for cn in ("ccq", "ssq", "cck", "ssk"):
            cs_sb[cn] = consts.tile([P, S], f16, name=f"cs_{cn}")
        arena = {}
        for h in range(NH):
            # one spare slot so the strided causal-mask band AP stays in range
            arena[h] = consts.tile([P, NST + 1, SC], f16, name=f"ar{h}")
        ones4 = consts.tile([P, 64], f16)
        nc.vector.memset(ones4, 1.0)
        den4 = consts.tile([P, SC], f32)
        nc.vector.memset(den4, 1.0)
        rcp4 = consts.tile([P, SC], f16)
        rcp_h3 = consts.tile([1, SC], f16)
        lnt = consts.tile([P, SC], f32)

        tmp = ctx.enter_context(tc.tile_pool(name="tmp", bufs=1))
        attnT_p = ctx.enter_context(tc.tile_pool(name="attnT", bufs=2))
        obp = ctx.enter_context(tc.tile_pool(name="obp", bufs=4))
        ps = ctx.enter_context(tc.tile_pool(name="ps", bufs=1, space="PSUM"))

        xTr = ap["xT"].rearrange("(a p) s -> p a s", p=P)
        wre = {wn: ap[wn].rearrange("(a p) m -> p a m", p=P)
               for wn in ("wq", "wk", "wv")}

        def dma_x_chunk(sc, eng):
            ssl = slice(sc * SC, (sc + 1) * SC)
            eng.dma_start(out=xT_sb[sc], in_=xTr[:, :, ssl])

        # ---- input DMAs: the first-needed tensors in fine pieces, split
        # across both rings so the prologue's a-loop is paced by arrivals.
        rings = (nc.scalar, nc.sync)
        for piece in range(4):
            asl = slice(2 * piece, 2 * piece + 2)
            rings[piece % 2].dma_start(out=w_sb["wq"][:, asl, :],
                                       in_=wre["wq"][:, asl, :])
            rings[(piece + 1) % 2].dma_start(out=xT_sb[0][:, asl, :],
                                             in_=xTr[:, asl, 0:SC])
        nc.scalar.dma_start(out=w_sb["wk"][:, 0:KT // 2, :],
                            in_=wre["wk"][:, 0:KT // 2, :])
        nc.sync.dma_start(out=w_sb["wk"][:, KT // 2:, :],
                          in_=wre["wk"][:, KT // 2:, :])
        nc.scalar.dma_start(out=w_sb["wv"], in_=wre["wv"])
        nc.sync.dma_start(out=pm_sb, in_=ap["pm"])
        for cn in ("ccq", "ssq", "cck", "ssk"):
            nc.scalar.dma_start(out=cs_sb[cn], in_=ap[cn])
        nc.sync.dma_start(out=tri4, in_=ap["tri4"].rearrange("p (a c) -> p a c", a=4))
        dma_x_chunk(1, nc.sync)
        nc.sync.dma_start(out=wo_sb, in_=ap["wo"].rearrange("(a p) m -> p a m", p=P))

        qk_specs = (("wq", "ccq", "ssq", qrot), ("wk", "cck", "ssk", krot))
        evs = {}

        # ---- stage1 pieces ------------------------------------------------
        def qk_fill(sc, wn, m, tag):
            pr = ps.tile([P, SC], f32, tag=tag, name=f"pr_{wn}{m}_{sc}")
            for a in range(KT):
                nc.tensor.matmul(
                    pr, w_sb[wn][:, a, m * P:(m + 1) * P], xT_sb[sc][:, a, :],
                    start=(a == 0), stop=(a == KT - 1))
            return pr

        def qk_drain(sc, wn, m, pr):
            ev = tmp.tile([P, SC], f16, tag=f"ev{wn}{m}", name=f"ev_{wn}{m}_{sc}")
            nc.vector.tensor_copy(ev, pr)
            evs[(wn, m)] = ev

        def sw_rope(sc, wn, m, tag):
            ssl = slice(sc * SC, (sc + 1) * SC)
            ccn, ssn, rot = {
                "wq": ("ccq", "ssq", qrot), "wk": ("cck", "ssk", krot)}[wn]
            ev = evs[(wn, m)]
            sw = ps.tile([P, SC], f32, tag=tag, name=f"sw_{wn}{m}_{sc}")
            nc.tensor.matmul(sw, pm_sb, ev, start=True, stop=True)
            t2 = tmp.tile([P, SC], f16, tag="t2", name=f"t2_{wn}{m}_{sc}", bufs=2)
            nc.vector.tensor_mul(t2, sw, cs_sb[ssn][:, ssl])
            t1 = tmp.tile([P, SC], f16, tag="t1", name=f"t1_{wn}{m}_{sc}", bufs=2)
            nc.gpsimd.tensor_mul(t1, ev, cs_sb[ccn][:, ssl])
            nc.vector.tensor_add(rot[sc][:, m, :], t1, t2)

        def v_fill(sc, st, tag):
            vp = ps.tile([P, DHG], f32, tag=tag, name=f"vp{sc}_{st}")
            for a in range(KT):
                nc.tensor.matmul(
                    vp, xT_sb[sc][:, a, st * P:(st + 1) * P], w_sb["wv"][:, a, :],
                    start=(a == 0), stop=(a == KT - 1))
            return vp

        def v_drain(sc, st, vp):
            nc.vector.tensor_copy(v65[sc][:, st, :, 0:HD],
                                  vp.rearrange("p (h d) -> p h d", h=NH))

        # ---- stage2 pieces ------------------------------------------------
        def sc2_unit(qc, t, ip):
            """Scores + exp for key tiles (ip, ip+1), heads 2t and 2t+1."""
            i0, i1 = ip, ip + 1
            start = max(i0 * P - qc * SC, 0)
            scrs = {}
            for h in (2 * t, 2 * t + 1):
                scrs[h] = ps.tile([P, 2, SC], f32, tag=f"scr{h % 2}",
                                  name=f"scr{h}_{qc}_{ip}")
            for j, i in ((0, i0), (1, i1)):
                for h in (2 * t, 2 * t + 1):
                    po = 64 * (h % 2)
                    nc.tensor.matmul(
                        scrs[h][:, j, start:SC],
                        krot[i // 4][po:po + 64, t, (i % 4) * P:(i % 4 + 1) * P],
                        qrot[qc][po:po + 64, t, start:SC],
                        start=True, stop=True)
            for h in (2 * t, 2 * t + 1):
                nc.scalar.activation(
                    arena[h][:, i0:i0 + 2, start:SC],
                    scrs[h][:, :, start:SC], Exp)

        def trimask(qc, h):
            """Zero the 4 causal triangles of chunk qc in head h's arena."""
            flat = arena[h].rearrange("p a c -> p (a c)")
            band = flat[:, 2048 * qc:2048 * qc + 2560]
            band = band.rearrange("p (a c) -> p a c", a=4)[:, :, 0:P]
            nc.gpsimd.tensor_mul(band, band, tri4)

        attnTs = {}

        def pv_fill(qc, h, lo, hi):
            nt = 4 * qc + 4
            pv = ps.tile([P, SC], f32, tag=f"pv{h % 2}", name=f"pv{h}_{qc}")
            for i in range(lo, hi):
                scol = max(i * P - qc * SC, 0)
                nc.tensor.matmul(
                    pv[0:65, scol:SC], v65[i // 4][:, i % 4, h, :],
                    arena[h][:, i, scol:SC],
                    start=(i == 0), stop=(i == nt - 1))
            return pv

        def pv_drain(qc, h, pv):
            attnT = attnTs[qc]
            nc.vector.tensor_copy(
                attnT[64 * (h % 2):64 * (h % 2) + 64, h // 2, :], pv[0:64, :])
            nc.scalar.copy(den4[32 * h:32 * h + 1, :], pv[64:65, :])

        def recip(qc):
            with nc.allow_low_precision(reason="f16 softmax denominators"):
                nc.vector.reciprocal(rcp4, den4)
            nc.vector.tensor_copy(rcp_h3, rcp4[96:97, :])

        def norm(qc, t):
            bcp = ps.tile([P, SC], f32, tag="misc", name=f"bcp{t}_{qc}")
            attnT = attnTs[qc]
            for par in range(2):
                h = 2 * t + par
                if h == 3:
                    lhs, rhs = ones4[0:1, :], rcp_h3
                else:
                    lhs = ones4[32 * h:32 * h + 1, :]
                    rhs = rcp4[32 * h:32 * h + 1, :]
                nc.tensor.matmul(bcp[64 * par:64 * par + 64, :],
                                 lhs, rhs, start=True, stop=True)
            for par in range(2):
                h = 2 * t + par
                sl = slice(64 * par, 64 * par + 64)
                nc.vector.tensor_mul(attnT[sl, t, :], attnT[sl, t, :], bcp[sl, :])

        def wo_unit(qc, ml, nn, tag):
            attnT = attnTs[qc]
            m = qc * (SC // P) + ml
            wo_ps = ps.tile([P, SC], f32, tag=tag, name=f"wo{m}_{nn}")
            for kd in range(2):
                nc.tensor.matmul(
                    wo_ps, attnT[:, kd, ml * P:(ml + 1) * P],
                    wo_sb[:, kd, nn * 512:(nn + 1) * 512],
                    start=(kd == 0), stop=(kd == 1))
            ob = obp.tile([P, 512], f16, tag="ob", name=f"ob{m}_{nn}")
            if nn == 0:
                nc.vector.tensor_copy(ob, wo_ps)
            else:
                nc.scalar.copy(ob, wo_ps)
            eng = (nc.sync, nc.gpsimd)[(ml + nn) % 2]
            eng.dma_start(
                out=out_ap[m * P:(m + 1) * P, nn * 512:(nn + 1) * 512], in_=ob)

        # ---- prologue: stage1(chunk 0), DMA-paced ------------------------
        # wq's two accumulation groups run a-tile-major (paced by the wq+x0
        # transfers), then wk's (paced by the wk transfer), on the score/pv
        # banks.  Swaps and the v projection follow on the stage1 banks.
        pros = {}
        for wn, tag0, tag1 in (("wq", "scr0", "scr1"), ("wk", "pv0", "pv1")):
            for m, tag in ((0, tag0), (1, tag1)):
                pros[(wn, m)] = ps.tile([P, SC], f32, tag=tag,
                                        name=f"pro_{wn}{m}")
            for a in range(KT):
                for m in range(2):
                    nc.tensor.matmul(
                        pros[(wn, m)], w_sb[wn][:, a, m * P:(m + 1) * P],
                        xT_sb[0][:, a, :], start=(a == 0), stop=(a == KT - 1))
            for m in range(2):
                qk_drain(0, wn, m, pros[(wn, m)])
        # rope swaps, then the chunk-0 v projection woven with the first
        # score pairs of chunk 0 (they only wait on the rope DVE adds)
        sw_rope(0, "wq", 0, "misc")
        sw_rope(0, "wq", 1, "s1qk")
        v_drain(0, 0, v_fill(0, 0, "misc"))
        sw_rope(0, "wk", 0, "s1qk")
        v_drain(0, 1, v_fill(0, 1, "misc"))
        sw_rope(0, "wk", 1, "s1qk")

        # ---- main phases --------------------------------------------------
        def s1_units(c):
            """Three role-based unit lists: q-side rope, k-side rope, v.
            The k rope must be complete by the end of part 2 so the next
            phase's first score matmuls never wait on it."""
            if c >= NSC:
                return [], [], []
            ua, ub, uc = [], [], []
            if c + 1 < NSC:
                ua.append(lambda c=c: dma_x_chunk(c + 1, nc.gpsimd))
            for wn, m, u in (("wq", 0, ua), ("wq", 1, ua),
                             ("wk", 0, ub), ("wk", 1, ub)):
                u.append(lambda c=c, wn=wn, m=m: qk_drain(
                    c, wn, m, qk_fill(c, wn, m, "s1qk")))
                u.append(lambda c=c, wn=wn, m=m: sw_rope(c, wn, m, "s1qk"))
            for st in range(4):
                uc.append(lambda c=c, st=st: v_drain(
                    c, st, v_fill(c, st, "misc")))
            return ua, ub, uc

        def sc2_units(qc, t):
            nt = 4 * qc + 4
            units = [lambda qc=qc, t=t, ip=ip: sc2_unit(qc, t, ip)
                     for ip in range(0, nt, 2)]
            units.append(lambda qc=qc, h=2 * t: trimask(qc, h))
            units.append(lambda qc=qc, h=2 * t + 1: trimask(qc, h))
            return units

        def pv_units(qc, t):
            nt = 4 * qc + 4
            units = []
            for h in (2 * t, 2 * t + 1):
                bnds = list(range(0, nt, 6)) + [nt]
                segs = [(bnds[i], bnds[i + 1]) for i in range(len(bnds) - 1)]
                holder = {}

                def seg_run(qc=qc, h=h, seg=None, holder=holder, last=False):
                    pv = holder.get("pv")
                    if pv is None:
                        pv = pv_fill(qc, h, seg[0], seg[1])
                        holder["pv"] = pv
                    else:
                        nt_ = 4 * qc + 4
                        for i in range(seg[0], seg[1]):
                            scol = max(i * P - qc * SC, 0)
                            nc.tensor.matmul(
                                pv[0:65, scol:SC], v65[i // 4][:, i % 4, h, :],
                                arena[h][:, i, scol:SC],
                                start=False, stop=(i == nt_ - 1))
                    if last:
                        pv_drain(qc, h, pv)
                for si, seg in enumerate(segs):
                    units.append(lambda seg=seg, f=seg_run,
                                 last=(si == len(segs) - 1): f(
                                     seg=seg, last=last))
            return units

        def wo_units(fc, tags=("misc",)):
            units = []
            for k, (ml, nn) in enumerate(
                    (ml, nn) for ml in range(4) for nn in range(2)):
                units.append(lambda fc=fc, ml=ml, nn=nn,
                             tag=tags[k % len(tags)]: wo_unit(fc, ml, nn, tag))
            return units

        # chunk-0 t=0 scores run in the prologue; thereafter each phase p
        # handles [scores(p,1); pv(p,0); pv(p,1); scores(p+1,0)] so the
        # exp/ACT load stays spread across phase boundaries.
        attnTs[0] = attnT_p.tile([P, 2, SC], f16, tag="at", name="at0")
        _weave(sc2_units(0, 0),
               [lambda: v_drain(0, 2, v_fill(0, 2, "misc")),
                lambda: v_drain(0, 3, v_fill(0, 3, "s1qk"))])

        for p in range(NSC):
            qc, c, fc = p, p + 1, p - 1
            if c < NSC:
                attnTs[c] = attnT_p.tile([P, 2, SC], f16, tag="at", name=f"at{c}")
            s1a, s1b, s1c = s1_units(c)
            wo = wo_units(fc, tags=("misc",) if c < NSC else ("misc", "s1qk")) \
                if fc >= 0 else []
            woa, wob = wo[:5], wo[5:]
            _weave(sc2_units(qc, 1), pv_units(qc, 0), s1a + s1b, woa)
            _weave(pv_units(qc, 1),
                   sc2_units(c, 0) if c < NSC else [],
                   s1c, wob)
            # normalize this chunk at the end of its own phase so the next
            # phase's wo never waits on a cold DVE queue
            if p < NSC - 1:
                recip(qc)
                norm(qc, 0)
                norm(qc, 1)
            else:
                with nc.allow_low_precision(reason="f16 softmax denoms"):
                    nc.vector.reciprocal(rcp4[0:64, :], den4[0:64, :])
                norm(qc, 0)

        # ---- epilogue: ACT-based reciprocal (exp(-ln)) for low latency ----
        Ln = mybir.ActivationFunctionType.Ln
        nc.scalar.activation(lnt[64:128, :], den4[64:128, :], Ln)
        with nc.allow_low_precision(reason="f16 softmax denominators"):
            nc.scalar.activation(rcp4[64:128, :], lnt[64:128, :], Exp, scale=-1.0)
        nc.vector.tensor_copy(rcp_h3, rcp4[96:97, :])
        norm(NSC - 1, 1)
        for th in wo_units(NSC - 1, tags=("misc", "s1qk")):
            th()

        if dbg is not None:
            for c in range(NSC):
                nc.sync.dma_start(out=dbg["qrot"][:, :, c * SC:(c + 1) * SC],
                                  in_=qrot[c])
                nc.sync.dma_start(out=dbg["krot"][:, :, c * SC:(c + 1) * SC],
                                  in_=krot[c])
                nc.sync.dma_start(out=dbg["v65"][:, 4 * c:4 * c + 4], in_=v65[c])
            nc.sync.dma_start(out=dbg["ar0"], in_=arena[0][:, 0:NST, :])
            nc.sync.dma_start(out=dbg["at3"], in_=attnTs[3])


def _build_program(debug=False):
    import concourse.tile as tile
    import concourse.mybir as mybir
    from concourse import bacc

    f16 = mybir.dt.float16

    nc = bacc.Bacc("TRN2", target_bir_lowering=False, debug=False,
                   num_devices=N_CORES)
    ap = {}

    def inp(name, shape, dt):
        ap[name] = nc.dram_tensor(name, shape, dt, kind="ExternalInput").ap()

    inp("xT", [D, S], f16)
    inp("wq", [D, DHG], f16)
    inp("wk", [D, DHG], f16)
    inp("wv", [D, DHG], f16)
    inp("wo", [DHG, D], f16)
    inp("ccq", [P, S], f16)
    inp("ssq", [P, S], f16)
    inp("cck", [P, S], f16)
    inp("ssk", [P, S], f16)
    inp("tri4", [P, 4 * P], f16)
    inp("pm", [P, P], f16)
    out_ap = nc.dram_tensor("out", [S, D], f16, kind="ExternalOutput").ap()
    dbg = None
    if debug:
        dbg = {
            "qrot": nc.dram_tensor("dbg_qrot", [P, 2, S], f16, kind="ExternalOutput").ap(),
            "krot": nc.dram_tensor("dbg_krot", [P, 2, S], f16, kind="ExternalOutput").ap(),
            "v65": nc.dram_tensor("dbg_v65", [P, NST, NH, HD + 1], f16, kind="ExternalOutput").ap(),
            "ar0": nc.dram_tensor("dbg_ar0", [P, NST, SC], f16, kind="ExternalOutput").ap(),
            "at3": nc.dram_tensor("dbg_at3", [P, 2, SC], f16, kind="ExternalOutput").ap(),
        }

    with tile.TileContext(nc) as tc:
        _emit(tc, nc, ap, out_ap, mybir, dbg=dbg)
    nc.compile()
    return nc


def _host_prep(x, wq, wk, wv, wo, freqs_cos, freqs_sin, mask):
    """Build the 8 per-core input maps."""
    perm = []
    for h in range(NH):
        perm += [HD * h + 2 * j for j in range(HD // 2)]
        perm += [HD * h + 2 * j + 1 for j in range(HD // 2)]
    perm = np.asarray(perm)

    cosT = np.ascontiguousarray(freqs_cos.T).astype(np.float32)   # [32, S]
    sinT = np.ascontiguousarray(freqs_sin.T).astype(np.float32)
    CC = np.tile(cosT, (4, 1))                                    # [128, S]
    SS = np.tile(np.vstack([-sinT, sinT]), (2, 1))                # [128, S]
    ccq, ssq = (CC * 0.125).astype(np.float16), (SS * 0.125).astype(np.float16)
    cck, ssk = CC.astype(np.float16), SS.astype(np.float16)

    swap = np.zeros((P, P), dtype=np.float16)
    for g in range(2):
        for j in range(32):
            swap[64 * g + 32 + j, 64 * g + j] = 1.0
            swap[64 * g + j, 64 * g + 32 + j] = 1.0

    m2 = mask[0, 0]
    # multiplicative 0/1 mask in [key, query] orientation, replicated 4x
    tri = (m2[0:P, 0:P].T > -0.5).astype(np.float16)
    tri4 = np.tile(tri, (1, 4)).astype(np.float16)

    xT = [np.ascontiguousarray(x[b].T).astype(np.float16) for b in range(B)]

    in_maps = []
    for c in range(N_CORES):
        b, hg = c // NHG, c % NHG
        cols = hg * DHG + np.arange(DHG)
        in_maps.append({
            "xT": xT[b],
            "wq": np.ascontiguousarray(wq[:, hg * DHG + perm]).astype(np.float16),
            "wk": np.ascontiguousarray(wk[:, hg * DHG + perm]).astype(np.float16),
            "wv": np.ascontiguousarray(wv[:, cols]).astype(np.float16),
            "wo": np.ascontiguousarray(wo[cols, :]).astype(np.float16),
            "ccq": ccq, "ssq": ssq, "cck": cck, "ssk": ssk,
            "tri4": tri4, "pm": swap,
        })
    return in_maps


def kernel(x, wq, wk, wv, wo, freqs_cos, freqs_sin, mask, start_pos=0, **_):
    import concourse.bass_utils as bass_utils

    x = np.asarray(x, dtype=np.float32)
    wq = np.asarray(wq, dtype=np.float32)
    wk = np.asarray(wk, dtype=np.float32)
    wv = np.asarray(wv, dtype=np.float32)
    wo = np.asarray(wo, dtype=np.float32)
    freqs_cos = np.asarray(freqs_cos, dtype=np.float32)
    freqs_sin = np.asarray(freqs_sin, dtype=np.float32)
    mask = np.asarray(mask, dtype=np.float32)

    trace = bool(int(os.environ.get("BASS_KERNEL_TRACE", "0")))
    if trace:
        _install_trace_shim()
        import concourse.bass_utils as bu
        bu.upload_artifacts = lambda tmpdir: "(upload skipped)"

    if "nc" not in _CACHE:
        _CACHE["nc"] = _build_program()
    nc = _CACHE["nc"]

    in_maps = _host_prep(x, wq, wk, wv, wo, freqs_cos, freqs_sin, mask)
    kwargs = {}
    if trace:
        kwargs = dict(trace=True, trace_cores=[0],
                      tmpdir=os.environ.get("BASS_KERNEL_TRACE_DIR", None))
    res = None
    last_exc = None
    for attempt in range(5):
        try:
            res = bass_utils.run_bass_kernel_spmd(
                nc, in_maps, core_ids=list(range(N_CORES)), **kwargs)
            break
        except Exception as e:  # transient NRT device errors recover on retry
            last_exc = e
            import time as _time
            _time.sleep(12)
    if res is None:
        raise last_exc
    _CACHE["last_result"] = res

    out = np.zeros((B, S, D), dtype=np.float32)
    for c in range(N_CORES):
        out[c // NHG] += res.results[c]["out"].astype(np.float32)
    return out
